# revision 3
# baseline (speedup 1.0000x reference)
"""Trainium2 Bass kernel for a causal self-attention block with LoRA adapters.

Model (B=2, T=2048, C=1024, H=16 heads, hd=64, LoRA r=32, scale 0.5):
    qkv = x @ w_attn.T + b_attn + 0.5*(x @ la_attn.T) @ lb_attn.T
    y   = causal_softmax_attention(q, k, v)
    out = y @ w_proj.T + b_proj + 0.5*(y @ la_proj.T) @ lb_proj.T

Sharding: Megatron-style tensor parallel over 8 NeuronCores. Each core owns
2 heads: column-split c_attn (its q/k/v rows), full attention for its heads,
row-split c_proj producing a partial-sum output; the host sums the 8 partials
(the "all-reduce") and transposes back.

Device algorithm per core (all matmuls bf16 with fp32 PSUM accumulation):
  - fold LoRA into effective weights on-device: W_eff = W + 0.5 * lb @ la
  - x.T resident in SBUF as bf16 [C, B*T] (host passes x.T, SWDGE casts)
  - qT/kT = W_qk_eff @ x.T   -> [256, 4096]  (channels on partitions)
  - v natural = x @ W_v_eff  -> [4096, 128]  (tokens on partitions)
  - per (batch, head): S.T[k, q] = kT.T @ qT blocks; P = exp(S/8) (no max
    subtraction; |S| < 3 for this distribution); causal mask via precomputed
    0/1 tiles; y.T = [v | 1].T @ P gives attention numerator + denominator
    in one accumulation; 1/denom = exp(-ln(denom)) on ScalarE; broadcast
    across partitions with a K=1 matmul; multiply.
  - outT_partial = W_proj_eff.T @ y_norm.T per head (K=64), + b_proj/8.
Output: bf16 partial [C, B*T] per core; host sums in fp32.
"""

from contextlib import ExitStack

import numpy as np
import ml_dtypes

import concourse.bass as bass
import concourse.tile as tile
from concourse import bacc, mybir
from concourse.bass_utils import run_bass_kernel_spmd

F32 = mybir.dt.float32
BF16 = mybir.dt.bfloat16
AF = mybir.ActivationFunctionType
ALU = mybir.AluOpType

B, T, C, H, R = 2, 2048, 1024, 16, 32
HD = C // H              # 64
NCORES = 8
HPC = H // NCORES        # 2 heads per core
CH = HPC * HD            # 128 per-core channels
BT = B * T               # 4096
NCT = C // 128           # 8 contraction tiles
NR = 3 * CH              # 384 qkv rows per core
KT = T // 128            # 16 key tiles per sequence
QCH = 512                # q chunk size
NQC = T // QCH           # 4 q chunks per sequence
TCH = 512                # token chunk for qkv/proj
NTC = BT // TCH          # 8

_CACHE: dict = {}


def _emit(ctx: ExitStack, tc: tile.TileContext, t_in: dict, outT):
    nc = tc.nc

    singles = ctx.enter_context(tc.tile_pool(name="singles", bufs=1))
    wst = ctx.enter_context(tc.tile_pool(name="wst", bufs=2))
    psA = ctx.enter_context(tc.tile_pool(name="psA", bufs=2, space=bass.MemorySpace.PSUM))
    psST = ctx.enter_context(tc.tile_pool(name="psST", bufs=2, space=bass.MemorySpace.PSUM))
    psY = ctx.enter_context(tc.tile_pool(name="psY", bufs=4, space=bass.MemorySpace.PSUM))
    ptp = ctx.enter_context(tc.tile_pool(name="ptp", bufs=6))
    ptx = ctx.enter_context(tc.tile_pool(name="ptx", bufs=3))
    yup = ctx.enter_context(tc.tile_pool(name="yup", bufs=4))
    dnp = ctx.enter_context(tc.tile_pool(name="dnp", bufs=2))
    outp = ctx.enter_context(tc.tile_pool(name="outp", bufs=4))

    # ---------- constants / weights to SBUF ----------
    la_sb = singles.tile([R, C], F32)
    nc.sync.dma_start(la_sb[:], t_in["la_attn"][:])
    lbq_sb = singles.tile([R, NR], F32)
    nc.sync.dma_start(lbq_sb[:], t_in["lbqkvT"][:])
    lapc_sb = singles.tile([R, HPC, HD], F32)
    nc.sync.dma_start(lapc_sb[:], t_in["lapc"][:])
    lbp_sb = singles.tile([R, C], F32)
    nc.sync.dma_start(lbp_sb[:], t_in["lbpT"][:])
    bq_sb = singles.tile([128, 3], F32)
    nc.sync.dma_start(bq_sb[:], t_in["b_qkv"][:].rearrange("(m p) -> p m", p=128))
    bp_sb = singles.tile([128, NCT], F32)
    nc.sync.dma_start(bp_sb[:], t_in["bp8"][:].rearrange("(m p) -> p m", p=128))
    bv_sb = singles.tile([1, CH], F32)
    nc.sync.dma_start(bv_sb[:], t_in["bv_row"][:])
    mask_sb = singles.tile([128, 4, QCH], BF16)
    nc.sync.dma_start(mask_sb[:], t_in["masks"][:].rearrange("r p q -> p r q"))
    ones_t = singles.tile([128, 128], F32)
    nc.vector.memset(ones_t[:], 1.0)

    # ---------- x.T -> SBUF bf16 (cast in DMA) ----------
    xb = singles.tile([128, NCT, BT], BF16)
    xT = t_in["xT"]
    for ct in range(NCT):
        for q4 in range(4):
            sl = slice(q4 * (BT // 4), (q4 + 1) * (BT // 4))
            nc.gpsimd.dma_start(xb[:, ct, sl], xT[ct * 128:(ct + 1) * 128, sl])

    # ---------- fold LoRA into effective weights ----------
    # qkv: W_effT[c, n] = wqkvT[c, n] + 0.5 * sum_r la[r, c] * lbqkvT[r, n]
    wq_eff = singles.tile([128, NCT, NR], BF16)
    for ct in range(NCT):
        w_raw = wst.tile([128, NR], F32, tag="wq_raw")
        nc.sync.dma_start(
            w_raw[:], t_in["wqkvT"][ct * 128:(ct + 1) * 128, :])
        f = psA.tile([128, NR], F32, tag="a")
        nc.tensor.matmul(f[:], la_sb[:, ct * 128:(ct + 1) * 128], lbq_sb[:],
                         start=True, stop=True)
        nc.vector.scalar_tensor_tensor(
            wq_eff[:, ct, :], f[:], 0.5, w_raw[:], ALU.mult, ALU.add)

    # proj (stored per-head, d on partitions 0..63):
    # wp_eff2[d, h, o] = wpT[h*64+d, o] + 0.5 * sum_r lapc[r, h, d] * lbpT[r, o]
    wp_eff = singles.tile([HD, HPC, C], BF16)
    for h in range(HPC):
        w_raw = wst.tile([HD, C], F32, tag="wp_raw")
        nc.sync.dma_start(w_raw[:], t_in["wpT"][h * HD:(h + 1) * HD, :])
        for half in range(2):
            f = psA.tile([HD, 512], F32, tag="a")
            nc.tensor.matmul(f[:], lapc_sb[:, h, :],
                             lbp_sb[:, half * 512:(half + 1) * 512],
                             start=True, stop=True)
            nc.vector.scalar_tensor_tensor(
                wp_eff[:, h, half * 512:(half + 1) * 512], f[:], 0.5,
                w_raw[:, half * 512:(half + 1) * 512], ALU.mult, ALU.add)

    # v bias broadcast across partitions: [128, CH]
    bvb_ps = psA.tile([128, CH], F32, tag="a")
    nc.tensor.matmul(bvb_ps[:], ones_t[0:1, :], bv_sb[:], start=True, stop=True)
    bvb = singles.tile([128, CH], F32)
    nc.vector.tensor_copy(bvb[:], bvb_ps[:])

    # ---------- qT / kT : [128(2 heads x 64), 2, BT] bf16 ----------
    qkT = singles.tile([128, 2, BT], BF16)
    for mt in range(2):
        for qc in range(NTC):
            sl = slice(qc * TCH, (qc + 1) * TCH)
            ps = psA.tile([128, TCH], F32, tag="a")
            for ct in range(NCT):
                nc.tensor.matmul(
                    ps[:], wq_eff[:, ct, mt * 128:(mt + 1) * 128],
                    xb[:, ct, sl], start=(ct == 0), stop=(ct == NCT - 1))
            nc.any.tensor_scalar(qkT[:, mt, sl], ps[:], bq_sb[:, mt:mt + 1],
                                 None, ALU.add)

    # ---------- v natural + ones column: v_ext [128, B, HPC, KT, 65] ----------
    v_ext = singles.tile([128, B, HPC, KT, HD + 1], BF16)
    nc.vector.memset(v_ext[:, :, :, :, HD:HD + 1], 1.0)
    for tt in range(BT // 128):
        b = (tt * 128) // T
        kt = (tt * 128 - b * T) // 128
        ps = psA.tile([128, CH], F32, tag="a")
        for ct in range(NCT):
            nc.tensor.matmul(
                ps[:], xb[:, ct, tt * 128:(tt + 1) * 128],
                wq_eff[:, ct, 2 * CH:3 * CH],
                start=(ct == 0), stop=(ct == NCT - 1))
        for h in range(HPC):
            nc.any.tensor_tensor(
                v_ext[:, b, h, kt, 0:HD], ps[:, h * HD:(h + 1) * HD],
                bvb[:, h * HD:(h + 1) * HD], ALU.add)

    # ---------- attention per (batch, head) ----------
    yn = singles.tile([HD, HPC, BT], BF16)  # normalized y.T per head
    for b in range(B):
        for h in range(HPC):
            hp = slice(h * HD, (h + 1) * HD)
            yps = [psY.tile([HD + 1, QCH], F32, tag="y", name=f"yps{b}_{h}_{j}")
                   for j in range(NQC)]
            for kt in range(KT):
                jlead = kt // 4
                r = kt % 4
                k_lhs = qkT[hp, 1, b * T + kt * 128: b * T + (kt + 1) * 128]
                for j in range(jlead, NQC):
                    cs = r * 128 if j == jlead else 0
                    q0 = b * T + j * QCH
                    st = psST.tile([128, QCH], F32, tag="st")
                    nc.tensor.matmul(st[:, cs:], k_lhs,
                                     qkT[hp, 0, q0 + cs: q0 + QCH],
                                     start=True, stop=True)
                    pt = ptp.tile([128, QCH], BF16, tag="pt")
                    if j == jlead:
                        pe = ptx.tile([128, QCH], BF16, tag="pe")
                        nc.scalar.activation(pe[:, cs:], st[:, cs:], AF.Exp,
                                             scale=0.125)
                        nc.vector.tensor_mul(pt[:, cs:], pe[:, cs:],
                                             mask_sb[:, r, cs:])
                    else:
                        nc.scalar.activation(pt[:, cs:], st[:, cs:], AF.Exp,
                                             scale=0.125)
                    nc.tensor.matmul(yps[j][:, cs:], v_ext[:, b, h, kt, :],
                                     pt[:, cs:], start=(kt == 0),
                                     stop=(kt == 4 * j + 3))
            for j in range(NQC):
                # stage numerator+denominator to SBUF, free PSUM
                yu = yup.tile([HD + 1, QCH], F32, tag="yu")
                nc.any.tensor_copy(yu[:], yps[j][:])
                # 1/denom via exp(-ln(denom)) on ScalarE (partition 64 lane)
                rc = dnp.tile([HD + 1, 2, QCH], F32, tag="rc")
                nc.scalar.activation(rc[HD:HD + 1, 0, :], yu[HD:HD + 1, :],
                                     AF.Ln)
                nc.scalar.activation(rc[HD:HD + 1, 1, :], rc[HD:HD + 1, 0, :],
                                     AF.Exp, scale=-1.0)
                # broadcast across 64 partitions with a K=1 matmul
                db = psA.tile([HD, QCH], F32, tag="a")
                nc.tensor.matmul(db[:], ones_t[HD:HD + 1, 0:HD],
                                 rc[HD:HD + 1, 1, :], start=True, stop=True)
                nc.vector.tensor_mul(
                    yn[:, h, b * T + j * QCH: b * T + (j + 1) * QCH],
                    yu[0:HD, :], db[:])

    # ---------- projection: outT_partial [C, BT] ----------
    for mt in range(NCT):
        for tc8 in range(NTC):
            sl = slice(tc8 * TCH, (tc8 + 1) * TCH)
            po = psA.tile([128, TCH], F32, tag="a")
            for h in range(HPC):
                nc.tensor.matmul(po[:], wp_eff[:, h, mt * 128:(mt + 1) * 128],
                                 yn[:, h, sl], start=(h == 0),
                                 stop=(h == HPC - 1))
            ot = outp.tile([128, TCH], BF16, tag="ot")
            nc.any.tensor_scalar(ot[:], po[:], bp_sb[:, mt:mt + 1], None,
                                 ALU.add)
            nc.sync.dma_start(outT[mt * 128:(mt + 1) * 128, sl], ot[:])


def _build():
    nc = bacc.Bacc("TRN2", target_bir_lowering=False, debug=False)
    t_in = {
        "xT": nc.dram_tensor("xT", [C, BT], F32, kind="ExternalInput"),
        "wqkvT": nc.dram_tensor("wqkvT", [C, NR], F32, kind="ExternalInput"),
        "lbqkvT": nc.dram_tensor("lbqkvT", [R, NR], F32, kind="ExternalInput"),
        "la_attn": nc.dram_tensor("la_attn", [R, C], F32, kind="ExternalInput"),
        "b_qkv": nc.dram_tensor("b_qkv", [NR], F32, kind="ExternalInput"),
        "wpT": nc.dram_tensor("wpT", [CH, C], F32, kind="ExternalInput"),
        "lapc": nc.dram_tensor("lapc", [R, HPC, HD], F32, kind="ExternalInput"),
        "lbpT": nc.dram_tensor("lbpT", [R, C], F32, kind="ExternalInput"),
        "bp8": nc.dram_tensor("bp8", [C], F32, kind="ExternalInput"),
        "bv_row": nc.dram_tensor("bv_row", [1, CH], F32, kind="ExternalInput"),
        "masks": nc.dram_tensor("masks", [4, 128, QCH], BF16, kind="ExternalInput"),
    }
    outT = nc.dram_tensor("outT", [C, BT], BF16, kind="ExternalOutput")
    t_in_aps = {k: v for k, v in t_in.items()}
    with tile.TileContext(nc) as tc:
        with ExitStack() as ctx:
            _emit(ctx, tc, t_in_aps, outT)
    nc.compile()
    return nc


def _make_in_maps(inputs: dict) -> list:
    f32 = np.float32
    x = np.asarray(inputs["x"], f32).reshape(BT, C)
    w_attn = np.asarray(inputs["w_attn"], f32)
    b_attn = np.asarray(inputs["b_attn"], f32)
    la_attn = np.ascontiguousarray(np.asarray(inputs["la_attn"], f32))
    lb_attn = np.asarray(inputs["lb_attn"], f32)
    w_proj = np.asarray(inputs["w_proj"], f32)
    b_proj = np.asarray(inputs["b_proj"], f32)
    la_proj = np.asarray(inputs["la_proj"], f32)
    lb_proj = np.asarray(inputs["lb_proj"], f32)

    xT = np.ascontiguousarray(x.T)                       # [C, BT]
    lbpT = np.ascontiguousarray(lb_proj.T)               # [R, C]

    # causal mask tiles: M[r, k, q] = 1 if q >= 128*r + k
    r_idx = np.arange(4)[:, None, None]
    k_idx = np.arange(128)[None, :, None]
    q_idx = np.arange(QCH)[None, None, :]
    masks = (q_idx >= 128 * r_idx + k_idx).astype(ml_dtypes.bfloat16)

    in_maps = []
    for core in range(NCORES):
        ch0 = core * CH
        rows = np.r_[ch0:ch0 + CH, C + ch0:C + ch0 + CH,
                     2 * C + ch0:2 * C + ch0 + CH]
        lapc = np.ascontiguousarray(
            la_proj[:, ch0:ch0 + CH].reshape(R, HPC, HD))
        in_maps.append({
            "xT": xT,
            "wqkvT": np.ascontiguousarray(w_attn[rows].T),
            "lbqkvT": np.ascontiguousarray(lb_attn[rows].T),
            "la_attn": la_attn,
            "b_qkv": np.ascontiguousarray(b_attn[rows]),
            "wpT": np.ascontiguousarray(w_proj[:, ch0:ch0 + CH].T),
            "lapc": lapc,
            "lbpT": lbpT,
            "bp8": np.ascontiguousarray(b_proj / NCORES),
            "bv_row": np.ascontiguousarray(b_attn[2 * C + ch0:2 * C + ch0 + CH]
                                           .reshape(1, CH)),
            "masks": masks,
        })
    return in_maps


def _execute(inputs: dict, trace: bool = False):
    if "nc" not in _CACHE:
        _CACHE["nc"] = _build()
    nc = _CACHE["nc"]
    in_maps = _make_in_maps(inputs)
    res = run_bass_kernel_spmd(nc, in_maps, core_ids=list(range(NCORES)),
                               trace=trace)
    acc = np.zeros((C, BT), np.float32)
    for r in res.results:
        acc += np.asarray(r["outT"], dtype=np.float32)
    out = np.ascontiguousarray(acc.T).reshape(B, T, C).astype(np.float32)
    return out, res


def kernel(**inputs) -> np.ndarray:
    out, _ = _execute(inputs, trace=False)
    return out
